# revision 14
# baseline (speedup 1.0000x reference)
"""Trainium2 Bass kernel for nn_EquivariantScalar (segment_reduce) — v7.

Redesign vs v5: six ACT-table eras (sqrt/silu alternating over chunk
halves) so the scalar engine never stalls the PE for long; squares/adds
spread across ACT/DVE/GPS by static assignment; gate+silu batched in
chunk pairs via a 2-bank ph pool; the scalar-output tail is computed with
a 13-column one-hot-embedded wcomb stationary (writes sf rows straight to
per-chunk PSUM partitions), DMA-xbar transposed, then reduced with 1-col
stationary matmuls against the mask.
"""
import sys

if "/opt/trn_rl_repo" not in sys.path:
    sys.path.insert(0, "/opt/trn_rl_repo")

import numpy as np
import ml_dtypes

import concourse.bass as bass
import concourse.mybir as mybir
import concourse.tile as tile
from concourse.tile_rust import add_dep_helper as tile_rust_add_dep
from concourse.bass_utils import run_bass_kernel_spmd

F = 128
B = 256
BM = 64
N_NODES = 50000
N_CORES = 8
NPC = N_NODES // N_CORES
PAD = 6272
CHUNK = 512
NCH = 13          # 12 x 512 + 1 x 128
NBLK = PAD // F   # 49
BF16 = mybir.dt.bfloat16
FP32 = mybir.dt.float32
AF = mybir.ActivationFunctionType
ALU = mybir.AluOpType

WARMUP_MM = 10

_CACHE = {}

WNAMES = ["w2aT", "m1asT", "m1avT", "m2ahiT", "w1aT", "w2bT", "mfoldT",
          "m1bvT"]
NW = len(WNAMES)

# era plan: halves so early eras only need the first DMA slabs
H1 = list(range(0, 8))
H2 = list(range(8, NCH))
ERA_PLAN = [
    ("sqA", H1), ("siA", H1),
    ("sqA", H2), ("sqB", H1),
    ("siA", H2), ("siB", H1),
    ("sqB", H2), ("siB", H2),
]


def _chunk(c):
    n0 = c * CHUNK
    return n0, min(CHUNK, PAD - n0)


def _pairs(chunks):
    out = []
    i = 0
    while i < len(chunks):
        if i + 1 < len(chunks) and chunks[i + 1] == chunks[i] + 1:
            out.append((chunks[i], chunks[i + 1]))
            i += 2
        else:
            out.append((chunks[i],))
            i += 1
    return out


def _quads(chunks):
    out = []
    i = 0
    while i < len(chunks):
        grp = [chunks[i]]
        while (len(grp) < 4 and i + len(grp) < len(chunks)
               and chunks[i + len(grp)] == grp[-1] + 1
               and _chunk(grp[-1])[1] == CHUNK):
            grp.append(chunks[i + len(grp)])
        out.append(grp)
        i += len(grp)
    return out


# route assignment for squares (PSUM readable only by ACT / DVE; DVE
# needs copy-then-square since both-PSUM tensor_tensor is illegal) and
# adds (GPS is SBUF-only). Tuned from traces.
def _sq_route(c, layer):
    # "ACT": Square activation straight from PSUM.
    # "DVE": tensor_copy to bf16 then 2x bf16 square on DVE.
    if layer == "A":
        return "DVE" if c % 3 == 2 else "ACT"
    return "DVE" if c % 3 == 1 else "ACT"


def _add_eng(c, layer, which):
    # which 0 = sqx+sqy, 1 = +sqz ; GPS takes the first add
    return "GPS" if which == 0 else "DVE"


def _build():
    nc = bass.Bass("TRN2", debug=False)

    sv_d = nc.dram_tensor("svT", (F, 4, PAD), BF16, kind="ExternalInput")
    m_d = nc.dram_tensor("mT", (F, NBLK, BM), BF16, kind="ExternalInput")
    w_d = nc.dram_tensor("wpack", (F, NW, F), BF16, kind="ExternalInput")
    wz_d = nc.dram_tensor("wz", (F, 32), BF16, kind="ExternalInput")
    b_d = nc.dram_tensor("bpack", (F, 4), FP32, kind="ExternalInput")
    y_d = nc.dram_tensor("y", (1, BM), FP32, kind="ExternalOutput")

    with nc.allow_low_precision(reason="bf16 intermediates are intentional"):
        with tile.TileContext(nc) as tc:
            with (
                tc.tile_pool(name="big", bufs=1) as big,
                tc.tile_pool(name="wk", bufs=2) as wk,
                tc.tile_pool(name="ps", bufs=1, space="PSUM") as ps,
            ):
                sv = big.tile([F, 4, PAD], BF16, name="sv_sb")
                wp = big.tile([F, NW, F], BF16, name="wp_sb")
                wzt = big.tile([F, 32], BF16, name="wz_sb")
                bp = big.tile([F, 4], FP32, name="bp_sb")
                mt = big.tile([F, NBLK, BM], BF16, name="mt_sb")

                nc.sync.dma_start(sv[:, :, 0:512], sv_d[:, :, 0:512])
                nc.sync.dma_start(wp[:], w_d[:])
                nc.sync.dma_start(wzt[:], wz_d[:])
                nc.sync.dma_start(bp[:], b_d[:])
                nc.sync.dma_start(sv[:, :, 512:1536], sv_d[:, :, 512:1536])
                nc.sync.dma_start(mt[:], m_d[:])
                nc.sync.dma_start(sv[:, :, 1536:2560], sv_d[:, :, 1536:2560])
                nc.sync.dma_start(sv[:, :, 2560:3584], sv_d[:, :, 2560:3584])
                nc.sync.dma_start(sv[:, :, 3584:4608], sv_d[:, :, 3584:4608])
                nc.sync.dma_start(sv[:, :, 4608:PAD], sv_d[:, :, 4608:PAD])

                h1r = big.tile([F, PAD], BF16, name="h1r")
                nsqA = big.tile([F, PAD], BF16, name="nsqA")
                v2na = big.tile([F, PAD], BF16, name="v2na")
                nsqB = big.tile([F, PAD], BF16, name="nsqB")
                v2nb = big.tile([F, PAD], BF16, name="v2nb")
                hbr = big.tile([F, PAD], BF16, name="hbr")
                sfsb = big.tile([32, CHUNK], BF16, name="sfsb")
                sfT = big.tile([F, 2, 4, 32], BF16, name="sfT")
                nc.vector.memset(sfsb[:, :], 0.0)

                W = {n: wp[:, i, :] for i, n in enumerate(WNAMES)}
                B1A = bp[:, 0:1]
                B2AHI = bp[:, 1:2]
                B1BE = bp[:, 2:3]
                ZERO = bp[:, 3:4]

                # ---- helpers ----------------------------------------
                last_act = [None]

                def act(*args, **kw):
                    inst = nc.scalar.activation(*args, **kw)
                    if last_act[0] is not None:
                        tile_rust_add_dep(inst.ins, last_act[0], sync=False,
                                          reason="act table-set ordering")
                    last_act[0] = inst.ins
                    return inst

                if WARMUP_MM:
                    pwarm = ps.tile([F, 2 * CHUNK], FP32, name="pwarm",
                                    tag="ph", bufs=1)
                    for i in range(WARMUP_MM):
                        nc.tensor.matmul(pwarm[:, 0:512], wp[:, 0, :],
                                         wp[:, 0:4, :])

                st = {c: {} for c in range(NCH)}

                def sq_engine_op(route, out, a, cp):
                    if route == "ACT":
                        act(out, a, AF.Square, bias=ZERO)
                    else:
                        nc.vector.tensor_copy(cp, a)
                        nc.vector.tensor_tensor(out, cp, cp, ALU.mult)

                # ---- era emitters -----------------------------------
                def emit_sq_chunk(c, layer):
                    # v2 matmuls + squares + adds for one chunk
                    n0, w = _chunk(c)
                    d = st[c]
                    wT = W["w2aT"] if layer == "A" else W["w2bT"]
                    if layer == "A":
                        vin = [sv[:, 1 + k, n0:n0 + w] for k in range(3)]
                    else:
                        vo = d.pop("vo")
                        vin = [vo[:, k, :w] for k in range(3)]
                    pxy = ps.tile([F, 2, CHUNK], FP32, name=f"pxy{layer}_{c}",
                                  tag="v3", bufs=2)
                    pz = ps.tile([F, CHUNK], FP32, name=f"pz{layer}_{c}",
                                 tag="vz", bufs=2)
                    nc.tensor.matmul(pxy[:, 0, :w], wT, vin[0])
                    nc.tensor.matmul(pxy[:, 1, :w], wT, vin[1])
                    nc.tensor.matmul(pz[:, :w], wT, vin[2])
                    sq = wk.tile([F, 3, CHUNK], BF16, name=f"sq{layer}_{c}",
                                 tag="sq", bufs=4)
                    route = _sq_route(c, layer)
                    if route == "DVE":
                        cp = wk.tile([F, 3, CHUNK], BF16,
                                     name=f"cp{layer}_{c}", tag="cp", bufs=2)
                    else:
                        cp = None
                    if w == CHUNK:
                        sq_engine_op(route, sq[:, 0:2, :], pxy[:],
                                     cp[:, 0:2, :] if cp is not None else None)
                    else:
                        sq_engine_op(route, sq[:, 0:2, :w], pxy[:, :, :w],
                                     cp[:, 0:2, :w] if cp is not None
                                     else None)
                    sq_engine_op(route, sq[:, 2, :w], pz[:, :w],
                                 cp[:, 2, :w] if cp is not None else None)
                    t01 = wk.tile([F, CHUNK], BF16, name=f"t01{layer}_{c}",
                                  tag="t01", bufs=4)
                    e0 = (nc.gpsimd if _add_eng(c, layer, 0) == "GPS"
                          else nc.vector)
                    e0.tensor_tensor(t01[:, :w], sq[:, 0, :w],
                                     sq[:, 1, :w], ALU.add)
                    nsq = nsqA if layer == "A" else nsqB
                    e1 = (nc.gpsimd if _add_eng(c, layer, 1) == "GPS"
                          else nc.vector)
                    e1.tensor_tensor(nsq[:, n0:n0 + w], t01[:, :w],
                                     sq[:, 2, :w], ALU.add)

                def emit_sqrt(chunks, layer):
                    nsq = nsqA if layer == "A" else nsqB
                    v2n = v2na if layer == "A" else v2nb
                    for grp in _quads(chunks):
                        lo = _chunk(grp[0])[0]
                        n1, w1 = _chunk(grp[-1])
                        hi = n1 + w1
                        act(v2n[:, lo:hi], nsq[:, lo:hi], AF.Sqrt, bias=ZERO)

                def emit_siA_pair(pair):
                    # ph1 mms -> silu -> phi mms -> gate; then per chunk:
                    # pvu mms, vo, v2b mms (+squares B emitted by caller)
                    php = ps.tile([F, 2 * CHUNK], FP32,
                                  name=f"ph1_{pair[0]}", tag="ph", bufs=1)
                    offs = []
                    for i, c in enumerate(pair):
                        n0, w = _chunk(c)
                        o = i * CHUNK
                        offs.append((c, n0, w, o))
                    for c, n0, w, o in offs:
                        nc.tensor.matmul(php[:, o:o + w], W["m1asT"],
                                         sv[:, 0, n0:n0 + w],
                                         start=True, stop=False)
                    for c, n0, w, o in offs:
                        nc.tensor.matmul(php[:, o:o + w], W["m1avT"],
                                         v2na[:, n0:n0 + w],
                                         start=False, stop=True)
                    if len(offs) == 2 and offs[0][2] == CHUNK:
                        n0 = offs[0][1]
                        wt = offs[0][2] + offs[1][2]
                        act(h1r[:, n0:n0 + wt], php[:, 0:wt], AF.Silu,
                            bias=B1A)
                    else:
                        for c, n0, w, o in offs:
                            act(h1r[:, n0:n0 + w], php[:, o:o + w], AF.Silu,
                                bias=B1A)
                    phip = ps.tile([F, 2 * CHUNK], FP32,
                                   name=f"phi_{pair[0]}", tag="ph", bufs=1)
                    for c, n0, w, o in offs:
                        nc.tensor.matmul(phip[:, o:o + w], W["m2ahiT"],
                                         h1r[:, n0:n0 + w])
                    gate = wk.tile([F, 2, CHUNK], BF16,
                                   name=f"gate_{pair[0]}", tag="gate", bufs=3)
                    if len(offs) == 2 and offs[0][2] == CHUNK:
                        nc.vector.tensor_scalar_add(gate[:], phip[:], B2AHI)
                    else:
                        for i, (c, n0, w, o) in enumerate(offs):
                            nc.vector.tensor_scalar_add(gate[:, i, :w],
                                                        phip[:, o:o + w],
                                                        B2AHI)
                    for i, (c, n0, w, o) in enumerate(offs):
                        pvxy = ps.tile([F, 2, CHUNK], FP32, name=f"pvxy_{c}",
                                       tag="v3", bufs=2)
                        pvz = ps.tile([F, CHUNK], FP32, name=f"pvz_{c}",
                                      tag="vz", bufs=2)
                        nc.tensor.matmul(pvxy[:, 0, :w], W["w1aT"],
                                         sv[:, 1, n0:n0 + w])
                        nc.tensor.matmul(pvxy[:, 1, :w], W["w1aT"],
                                         sv[:, 2, n0:n0 + w])
                        nc.tensor.matmul(pvz[:, :w], W["w1aT"],
                                         sv[:, 3, n0:n0 + w])
                        vo = wk.tile([F, 3, CHUNK], BF16, name=f"vo_{c}",
                                     tag="vo", bufs=3)
                        g1 = gate[:, i, :w]
                        g2 = g1.unsqueeze(1).broadcast_to((F, 2, w))
                        nc.vector.tensor_tensor(vo[:, 0:2, :w],
                                                pvxy[:, :, :w], g2, ALU.mult)
                        nc.vector.tensor_tensor(vo[:, 2, :w], pvz[:, :w], g1,
                                                ALU.mult)
                        st[c]["vo"] = vo
                        emit_sq_chunk(c, "B")

                tailst = {"y": None, "nmm": 0, "sfps": None}

                def emit_tail_flush(done_chunks, half, last=False):
                    # copy new sf rows, transpose, run the ready y-mms
                    lo, hi = done_chunks[0], done_chunks[-1]
                    sfps = tailst["sfps"]
                    nc.vector.tensor_copy(sfsb[0:16, :], sfps[0:16, :])
                    for k in range(4):
                        nc.sync.dma_start_transpose(
                            sfT[:, half, k, :],
                            sfsb[0:32, k * F:(k + 1) * F])
                    blocks = [j for j in range(NBLK)
                              if lo <= j // 4 <= hi
                              and (j // 4 < NCH - 1 or j % 4 == 0)]
                    if tailst["y"] is None:
                        tailst["y"] = ps.tile([F, CHUNK], FP32,
                                              name="y_ps", tag="vz", bufs=2)
                    for j in blocks:
                        c, k = j // 4, j % 4
                        stop = last and j == blocks[-1]
                        nc.tensor.matmul(tailst["y"][0:1, 0:BM],
                                         sfT[:, half, k, c:c + 1],
                                         mt[:, j, :],
                                         start=tailst["nmm"] == 0, stop=stop,
                                         skip_group_check=True)
                        tailst["nmm"] += 1

                def emit_siB_pair(pair):
                    if tailst["sfps"] is None:
                        tailst["sfps"] = ps.tile([F, CHUNK], FP32,
                                                 name="sfps", tag="vz",
                                                 bufs=2)
                        nc.vector.memset(tailst["sfps"][0:16, :], 0.0)
                    phb = ps.tile([F, 2 * CHUNK], FP32,
                                  name=f"phb_{pair[0]}", tag="ph", bufs=1)
                    offs = []
                    for i, c in enumerate(pair):
                        n0, w = _chunk(c)
                        offs.append((c, n0, w, i * CHUNK))
                    for c, n0, w, o in offs:
                        nc.tensor.matmul(phb[:, o:o + w], W["mfoldT"],
                                         h1r[:, n0:n0 + w],
                                         start=True, stop=False)
                    for c, n0, w, o in offs:
                        nc.tensor.matmul(phb[:, o:o + w], W["m1bvT"],
                                         v2nb[:, n0:n0 + w],
                                         start=False, stop=True)
                    if len(offs) == 2 and offs[0][2] == CHUNK:
                        n0 = offs[0][1]
                        wt = offs[0][2] + offs[1][2]
                        act(hbr[:, n0:n0 + wt], phb[:, 0:wt], AF.Silu,
                            bias=B1BE)
                    else:
                        for c, n0, w, o in offs:
                            act(hbr[:, n0:n0 + w], phb[:, o:o + w], AF.Silu,
                                bias=B1BE)
                    for c, n0, w, o in offs:
                        nc.tensor.matmul(tailst["sfps"][0:NCH, 0:w],
                                         wzt[:, NCH - c:2 * NCH - c],
                                         hbr[:, n0:n0 + w],
                                         start=c == 0, stop=c == NCH - 1,
                                         skip_group_check=True)

                # ---- main emission ----------------------------------
                for kind, chunks in ERA_PLAN:
                    if kind == "sqA":
                        for c in chunks:
                            emit_sq_chunk(c, "A")
                        emit_sqrt(chunks, "A")
                    elif kind == "sqB":
                        emit_sqrt(chunks, "B")
                    elif kind == "siA":
                        for pair in _pairs(chunks):
                            emit_siA_pair(pair)
                    elif kind == "siB":
                        for pair in _pairs(chunks):
                            emit_siB_pair(pair)
                        if chunks[-1] == H1[-1]:
                            emit_tail_flush(H1, 0)
                        elif chunks[-1] == NCH - 1:
                            emit_tail_flush(H2, 1, last=True)

                y_sb = wk.tile([1, BM], FP32, name="y_sb", tag="ysb")
                nc.vector.tensor_copy(y_sb[:], tailst["y"][0:1, 0:BM])
                nc.sync.dma_start(y_d[:], y_sb[:])

    _dedupe_ldweights(nc)
    _split_sync_waits_inline(nc, max_waits=1)
    return nc


def _dedupe_ldweights(nc):
    f = nc.m.functions[0]
    removed = 0
    for blk in f.blocks:
        new_insts = []
        last_sig = None
        pending_waits = []
        for inst in blk.instructions:
            tn = type(inst).__name__
            if getattr(inst, "engine", None) != mybir.EngineType.PE:
                new_insts.append(inst)
                continue
            if tn == "InstLdweights":
                ap = inst.ins[0]
                sig = (ap.memref, ap.offset, str(ap.ap), str(ap.dtype),
                       str(getattr(inst, "perf_mode", None)))
                if sig == last_sig:
                    si = inst.sync_info
                    if si is not None:
                        pending_waits.extend(si.on_wait or [])
                        assert not si.on_update
                    removed += 1
                    continue
                last_sig = sig
            elif tn == "InstMatmult":
                if getattr(inst, "is_transpose", False):
                    last_sig = None
            if pending_waits:
                si = inst.sync_info
                old_w = list(si.on_wait) if si and si.on_wait else []
                old_u = list(si.on_update) if si and si.on_update else []
                inst.sync_info = mybir.SyncInfo(
                    on_wait=pending_waits + old_w, on_update=old_u)
                pending_waits = []
            new_insts.append(inst)
        assert not pending_waits
        blk.instructions[:] = new_insts
    return removed


def _split_sync_waits_inline(nc, max_waits=1):
    f = nc.m.functions[0]
    counter = [0]
    for blk in f.blocks:
        new_insts = []
        for inst in blk.instructions:
            si = getattr(inst, "sync_info", None)
            waits = list(si.on_wait) if si and si.on_wait else []
            if len(waits) > max_waits:
                head, rest = waits[:-max_waits], waits[-max_waits:]
                for i in range(0, len(head), max_waits):
                    counter[0] += 1
                    nop = mybir.InstNoOp(
                        name=f"I-wsplit-{counter[0]}",
                        engine=inst.engine,
                        ins=[],
                        outs=[],
                        sync_info=mybir.SyncInfo(
                            on_wait=head[i:i + max_waits], on_update=[]),
                    )
                    new_insts.append(nop)
                inst.sync_info = mybir.SyncInfo(on_wait=rest,
                                                on_update=list(si.on_update))
            new_insts.append(inst)
        blk.instructions[:] = new_insts


def _get_nc():
    if "nc" not in _CACHE:
        _CACHE["nc"] = _build()
    return _CACHE["nc"]


def _prep_inputs(s, v, batch_mask, w1, w2, mlp_w1, mlp_b1, mlp_w2, mlp_b2,
                 out_w, out_b):
    bf16 = ml_dtypes.bfloat16
    s = np.asarray(s, np.float32)
    v = np.asarray(v, np.float32)
    batch_mask = np.asarray(batch_mask, np.float32)

    w1 = np.asarray(w1, np.float32)
    w2 = np.asarray(w2, np.float32)
    mlp_w1 = np.asarray(mlp_w1, np.float32)
    mlp_b1 = np.asarray(mlp_b1, np.float32)
    mlp_w2 = np.asarray(mlp_w2, np.float32)
    mlp_b2 = np.asarray(mlp_b2, np.float32)
    out_w = np.asarray(out_w, np.float32)
    out_b = np.asarray(out_b, np.float32)

    m1bs = mlp_w1[1][:, :F]
    wfold = m1bs @ mlp_w2[0][:F, :]
    b1b_eff = mlp_b1[1] + m1bs @ mlp_b2[0][:F]
    wcomb = out_w[0] @ mlp_w2[1][:F, :]
    bconst = float(out_w[0] @ mlp_b2[1][:F] + out_b[0])

    wmats = {
        "w2aT": w2[0].T, "m1asT": mlp_w1[0][:, :F].T,
        "m1avT": mlp_w1[0][:, F:].T, "m2ahiT": mlp_w2[0][F:, :].T,
        "w1aT": w1[0].T, "w2bT": w2[1].T, "mfoldT": wfold.T,
        "m1bvT": mlp_w1[1][:, F:].T,
    }
    wpack = np.zeros((F, NW, F), np.float32)
    for i, n in enumerate(WNAMES):
        wpack[:, i, :] = wmats[n]
    wz = np.zeros((F, 32), np.float32)
    wz[:, NCH] = wcomb
    bpack = np.zeros((F, 4), np.float32)
    bpack[:, 0] = mlp_b1[0]
    bpack[:, 1] = mlp_b2[0][F:]
    bpack[:, 2] = b1b_eff

    shared = {
        "wpack": np.ascontiguousarray(wpack.astype(bf16)),
        "wz": np.ascontiguousarray(wz.astype(bf16)),
        "bpack": np.ascontiguousarray(bpack),
    }

    mask_nb = batch_mask[:, :, 0].T
    mol_of_atom = np.argmax(mask_nb, axis=1)
    in_maps = []
    mol_lo = []
    for k in range(N_CORES):
        lo, hi = k * NPC, (k + 1) * NPC
        m0 = int(mol_of_atom[lo])
        m1 = int(mol_of_atom[hi - 1])
        assert m1 - m0 + 1 <= BM, f"core {k}: {m1 - m0 + 1} molecules > {BM}"
        mol_lo.append(m0)
        sk = np.zeros((PAD, F), np.float32)
        sk[:NPC] = s[0, lo:hi]
        vk = np.zeros((PAD, 3, F), np.float32)
        vk[:NPC] = v[0, lo:hi]
        mk = np.zeros((PAD, BM), np.float32)
        msl = mask_nb[lo:hi, m0:min(m0 + BM, B)]
        mk[:NPC, :msl.shape[1]] = msl
        m = dict(shared)
        svk = np.empty((F, 4, PAD), np.float32)
        svk[:, 0, :] = sk.T
        svk[:, 1:4, :] = vk.transpose(2, 1, 0)
        m["svT"] = np.ascontiguousarray(svk.astype(bf16))
        m["mT"] = np.ascontiguousarray(
            mk.reshape(NBLK, F, BM).transpose(1, 0, 2).astype(bf16))
        in_maps.append(m)
    cnt = batch_mask[:, :, 0].sum(axis=1)
    return in_maps, mol_lo, bconst, cnt


def run(inputs, trace=False, **kw):
    nc = _get_nc()
    in_maps, mol_lo, bconst, cnt = _prep_inputs(
        inputs["s"], inputs["v"], inputs["batch_mask"], inputs["w1"],
        inputs["w2"], inputs["mlp_w1"], inputs["mlp_b1"], inputs["mlp_w2"],
        inputs["mlp_b2"], inputs["out_w"], inputs["out_b"])
    res = run_bass_kernel_spmd(nc, in_maps, list(range(N_CORES)),
                               trace=trace, **kw)
    y = np.zeros(B, np.float64)
    for k in range(N_CORES):
        yk = res.results[k]["y"].astype(np.float64).reshape(BM)
        m0 = mol_lo[k]
        nb = min(BM, B - m0)
        y[m0:m0 + nb] += yk[:nb]
    y += np.float64(bconst) * cnt.astype(np.float64)
    return y.astype(np.float32).reshape(B, 1), res


def kernel(**inputs):
    y, _ = run(inputs)
    return y
